# revision 19
# baseline (speedup 1.0000x reference)
"""BigBird ViT forward on 8 Trainium2 NeuronCores.

Sharding: 2 groups of 4 cores (one per batch element). Within a group,
tokens are sharded 4-way (272 of the 1088 padded tokens per core) for all
dense matmuls / layernorms (weights replicated, streamed from HBM in bf16),
and attention is computed for the core's own 272 query tokens over all 12
heads, after a per-layer AllGather of K^T and V split in two halves by
head group (heads 0-5 / 6-11) so the second gather overlaps attention on
the first half.

Everything on-chip lives transposed ([feature, token]) so the PE contracts
over partitions without any activation transposes; LayerNorm reductions over
the feature dim use ones-vector matmuls in bf16 on the PE.

The BigBird band/random/global structure (plus seq padding) is applied as a
multiplicative {0,1,2} bf16 mask on the unnormalized attention probabilities;
with S=1025 the reference's -10000 additive masking underflows exp() to
exactly 0, so this is an exact reformulation. Keys are truncated to the real
1025 (k-tiles 0-7 = keys 0..1023 plus a rank-1 tile for key 1024).
"""
import os
import sys

sys.path.insert(0, "/opt/trn_rl_repo")

import numpy as np
import ml_dtypes

import concourse.bass as bass
import concourse.bacc as bacc
import concourse.mybir as mybir
import concourse.tile as tile
from concourse.bass_utils import run_bass_kernel_spmd

F32 = mybir.dt.float32
BF16 = mybir.dt.bfloat16
AF = mybir.ActivationFunctionType
ALU = mybir.AluOpType
BF = ml_dtypes.bfloat16

# model dims
BS = 64; NH = 12; HD = 64; D = 768; F = 3072; L = 12; R = 3
SEQ = 1025
SEQP = 1088           # padded to 17 blocks of 64
NBLK = 17
T = SEQP // 4         # tokens per core = 272
DT = D // 128         # 6 feature tiles
FT = F // 128         # 24 ffn tiles
KT = 9                # conceptual k tiles; tiles 0-7 full, tile 8 = key 1024
KPAD = 1152           # Ktf column stride per feature tile
VCOLS = NH * (HD + 1)  # 780: per-head [64 V cols + 1 ones col]
VH = VCOLS // 2        # 390 (heads 0-5 | 6-11)
SC = 1.0 / np.sqrt(HD)

NLAYERS = int(os.environ.get("BB_NLAYERS", str(L)))
# partition_broadcast crashes and reciprocal_approx_fast mis-computes on this
# runtime (missing gpsimd / custom-DVE ucode); use PE broadcast + exact recip.
NO_EXP3 = bool(os.environ.get("BB_NO_EXP3"))   # avoid 3D-AP batched exp
NO_GPDMA = bool(os.environ.get("BB_NO_GPDMA")) # avoid gpsimd-issued DMAs

_CACHE = {}

RG = [[0, 1, 2, 3], [4, 5, 6, 7]]


# ---------------------------------------------------------------- builder

def build_program(nlayers=NLAYERS):
    nc = bacc.Bacc("TRN2", target_bir_lowering=False, debug=False, num_devices=8)

    # ---- DRAM I/O -------------------------------------------------------
    pe_in = nc.dram_tensor("pe_in", [128, DT * T], BF16, kind="ExternalInput")
    add_in = nc.dram_tensor("add_in", [128, DT * T], F32, kind="ExternalInput")
    mask_in = nc.dram_tensor("mask_in", [128, NH * 8 * T], BF16, kind="ExternalInput")
    pw_in = nc.dram_tensor("pw", [D, D], BF16, kind="ExternalInput")
    normp_in = nc.dram_tensor("normp", [128, 2 * DT], F32, kind="ExternalInput")
    wq = [nc.dram_tensor(f"wq{i}", [D, D], BF16, kind="ExternalInput") for i in range(nlayers)]
    wk = [nc.dram_tensor(f"wk{i}", [D, D], BF16, kind="ExternalInput") for i in range(nlayers)]
    wv = [nc.dram_tensor(f"wv{i}", [D, VCOLS], BF16, kind="ExternalInput") for i in range(nlayers)]
    wo = [nc.dram_tensor(f"wo{i}", [D, D], BF16, kind="ExternalInput") for i in range(nlayers)]
    # w1/w2 shipped pre-tiled o-major: [128, OT*CT*128] with each 128x128 tile
    # contiguous, all contraction tiles of one output tile adjacent.
    w1 = [nc.dram_tensor(f"w1{i}", [128, FT * D], BF16, kind="ExternalInput") for i in range(nlayers)]
    w2 = [nc.dram_tensor(f"w2{i}", [128, DT * F], BF16, kind="ExternalInput") for i in range(nlayers)]
    lnp = [nc.dram_tensor(f"lnp{i}", [128, 72], F32, kind="ExternalInput") for i in range(nlayers)]
    bvb = [nc.dram_tensor(f"bvb{i}", [128, VCOLS], BF16, kind="ExternalInput") for i in range(nlayers)]
    esel_in = nc.dram_tensor("esel", [NH, DT * 128], BF16, kind="ExternalInput")
    out_t = nc.dram_tensor("out", [128, DT * T], BF16, kind="ExternalOutput")

    # collective bounce buffers (internal DRAM), reused across layers.
    # Each half: K^T for 3 feature tiles + V cols for 6 heads.
    KH_K = 3 * 128 * T
    KH_V = T * VH
    KVH = KH_K + KH_V
    kva_in = nc.dram_tensor("kva_in", [KVH], BF16)
    kvb_in = nc.dram_tensor("kvb_in", [KVH], BF16)
    kva_out = nc.dram_tensor("kva_out", [4 * KVH], BF16)
    kvb_out = nc.dram_tensor("kvb_out", [4 * KVH], BF16)

    with tile.TileContext(nc) as tc:
        # ---- persistent SBUF tensors -----------------------------------
        X = nc.alloc_sbuf_tensor("X", [128, DT * T], F32)          # residual, ft-tile major
        Xbf = nc.alloc_sbuf_tensor("Xbf", [128, DT * T], BF16)     # bf16 copy for LN stats
        xn = nc.alloc_sbuf_tensor("xn", [128, DT * T], BF16)       # LN output
        Qt = nc.alloc_sbuf_tensor("Qt", [128, DT * T], BF16)       # Q^T local
        Ktl = nc.alloc_sbuf_tensor("Ktl", [128, DT * T], BF16)     # K^T local
        Vnl = nc.alloc_sbuf_tensor("Vnl", [128, 3 * VCOLS], BF16)  # V-nat local (3 tok tiles)
        Ktf = nc.alloc_sbuf_tensor("Ktf", [128, DT * KPAD], BF16)  # K^T full
        Vnf = nc.alloc_sbuf_tensor("Vnf", [128, KT * VCOLS], BF16)  # V-nat full
        ctx = nc.alloc_sbuf_tensor("ctx", [128, DT * T], BF16)     # attention out^T
        hsb = nc.alloc_sbuf_tensor("hsb", [128, FT * T], BF16)     # ffn hidden^T
        msk = nc.alloc_sbuf_tensor("msk", [128, NH * 8 * T], BF16)
        onesb = nc.alloc_sbuf_tensor("onesb", [128, 1], BF16)      # for LN stat matmuls
        ones1 = nc.alloc_sbuf_tensor("ones1", [1, 128], BF16)      # fallback PE bcast
        ones1f = nc.alloc_sbuf_tensor("ones1f", [1, 128], F32)
        mu_b = nc.alloc_sbuf_tensor("mu_b", [128, T], BF16)
        rs_b = nc.alloc_sbuf_tensor("rs_b", [128, T], BF16)
        epsb = nc.alloc_sbuf_tensor("epsb", [128, 1], F32)
        zall = nc.alloc_sbuf_tensor("zall", [NH, T], F32)          # per-head softmax Z
        esel_sb = nc.alloc_sbuf_tensor("esel_sb", [NH, DT * 128], BF16)

        with (
            tc.tile_pool(name="wp", bufs=1) as wp,              # weight slabs (tags w/ bufs)
            tc.tile_pool(name="work", bufs=2) as work,
            tc.tile_pool(name="stat", bufs=2) as stat,
            tc.tile_pool(name="ppool", bufs=3) as ppool,        # P tiles per head
            tc.tile_pool(name="ps", bufs=2, space="PSUM") as ps,       # [128,1536] slots
            tc.tile_pool(name="psc", bufs=2, space="PSUM") as psc,     # PV accum + p8 + LN stats
        ):
            nc.vector.memset(onesb[:], 1.0)
            nc.vector.memset(ones1[:], 1.0)
            nc.vector.memset(ones1f[:], 1.0)
            nc.vector.memset(epsb[:], 1e-5)

            kvdma = nc.sync if NO_GPDMA else nc.gpsimd

            def bcast(dst, src_row):
                """broadcast [1,T] src_row to dst [P,T] via K=1 PE matmul."""
                p = dst.partition_size()
                one = ones1f if src_row.dtype == F32 else ones1
                bps = ps.tile([128, 1536], F32, tag="mm")
                nc.tensor.matmul(bps[:p, 0:T], one[:, 0:p], src_row,
                                 start=True, stop=True)
                nc.vector.tensor_copy(out=dst, in_=bps[:p, 0:T])
            # load masks + Z selector (resident)
            nc.sync.dma_start(out=msk[:], in_=mask_in[:, :])
            nc.sync.dma_start(out=esel_sb[:], in_=esel_in[:, :])

            def layernorm(src_f32, gcol, out_bf):
                """src [128, DT*T] f32 ft-major -> out bf16, LN over features."""
                # bf16 copy for cheap stat matmuls (1 cycle/row vs 4 for f32)
                for t in range(DT):
                    nc.vector.tensor_copy(
                        out=Xbf[:, t * T:(t + 1) * T], in_=src_f32[:, t * T:(t + 1) * T])
                sum_ps = psc.tile([1, T], F32, tag="ctx")
                for t in range(DT):
                    nc.tensor.matmul(sum_ps[:], onesb[:], Xbf[:, t * T:(t + 1) * T],
                                     start=(t == 0), stop=(t == DT - 1))
                mu = stat.tile([1, T], BF16, tag="mu")
                nc.scalar.activation(mu[:], sum_ps[:], AF.Identity, scale=1.0 / D)
                sq_ps = psc.tile([1, T], F32, tag="ctx")
                for t in range(DT):
                    sq = work.tile([128, T], BF16, tag="lnsq")
                    nc.vector.tensor_mul(sq[:], Xbf[:, t * T:(t + 1) * T],
                                         Xbf[:, t * T:(t + 1) * T])
                    nc.tensor.matmul(sq_ps[:], onesb[:], sq[:],
                                     start=(t == 0), stop=(t == DT - 1))
                var = stat.tile([1, T], F32, tag="var")
                nc.scalar.activation(var[:], sq_ps[:], AF.Identity, scale=1.0 / D)
                mu2 = stat.tile([1, T], F32, tag="mu2")
                nc.vector.tensor_mul(mu2[:], mu[:], mu[:])
                nc.vector.tensor_sub(var[:], var[:], mu2[:])
                std = stat.tile([1, T], F32, tag="std")
                nc.scalar.activation(std[:], var[:], AF.Sqrt, bias=epsb[0:1, 0:1])
                rstd32 = stat.tile([1, T], F32, tag="rstd32")
                nc.vector.reciprocal(rstd32[:], std[:])
                rstd = stat.tile([1, T], BF16, tag="rstd")
                nc.vector.tensor_copy(out=rstd[:], in_=rstd32[:])
                # broadcast mu, rstd to [128, T]
                bcast(mu_b[:], mu[0:1, :])
                bcast(rs_b[:], rstd[0:1, :])
                g = lnp_sb[:, gcol:gcol + DT]
                b = lnp_sb[:, gcol + DT:gcol + 2 * DT]
                for t in range(DT):
                    tmp = work.tile([128, T], BF16, tag="lnt")
                    nc.vector.tensor_sub(tmp[:], src_f32[:, t * T:(t + 1) * T], mu_b[:])
                    nc.vector.tensor_mul(tmp[:], tmp[:], rs_b[:])
                    nc.vector.tensor_scalar(
                        out_bf[:, t * T:(t + 1) * T], tmp[:],
                        g[:, t:t + 1], b[:, t:t + 1], op0=ALU.mult, op1=ALU.add)

            def load_slabs(wdram, tag, ncols):
                slabs = []
                for t in range(DT):
                    slab = wp.tile([128, 780], BF16, tag=tag, bufs=6)
                    nc.sync.dma_start(out=slab[:, :ncols],
                                      in_=wdram[t * 128:(t + 1) * 128, :])
                    slabs.append(slab)
                return slabs

            def proj_out(slabs, src_bf, out_bf, bias_col, orange):
                """out^T tiles (o in orange) = w^T @ src + bias, evac on DVE."""
                for o in orange:
                    psm = ps.tile([128, 1536], F32, tag="mm")
                    for t in range(DT):
                        nc.tensor.matmul(psm[:, 0:T], slabs[t][:, o * 128:(o + 1) * 128],
                                         src_bf[:, t * T:(t + 1) * T],
                                         start=(t == 0), stop=(t == DT - 1))
                    nc.vector.tensor_scalar(
                        out_bf[:, o * T:(o + 1) * T], psm[:, 0:T],
                        lnp_sb[:, bias_col + o:bias_col + o + 1], None, op0=ALU.add)

            # ---- embedding -------------------------------------------------
            eslabs = load_slabs(pw_in, "qslab", D)
            for o in range(DT):
                psm = ps.tile([128, 1536], F32, tag="mm")
                for t in range(DT):
                    peint = work.tile([128, T], BF16, tag="peint")
                    nc.sync.dma_start(out=peint[:], in_=pe_in[:, t * T:(t + 1) * T])
                    nc.tensor.matmul(psm[:, 0:T], eslabs[t][:, o * 128:(o + 1) * 128],
                                     peint[:], start=(t == 0), stop=(t == DT - 1))
                addt = work.tile([128, T], F32, tag="addt")
                nc.sync.dma_start(out=addt[:], in_=add_in[:, o * T:(o + 1) * T])
                nc.vector.tensor_add(X[:, o * T:(o + 1) * T], psm[:, 0:T], addt[:])

            kinA = kva_in[:KH_K].rearrange("(a b) -> a b", b=T)       # [384, T]
            vinA = kva_in[KH_K:].rearrange("(a b) -> a b", b=VH)      # [T, 390]
            kinB = kvb_in[:KH_K].rearrange("(a b) -> a b", b=T)
            vinB = kvb_in[KH_K:].rearrange("(a b) -> a b", b=VH)

            def bounce_half(kin, vin, tlo, nlo):
                kin3 = kin.rearrange("(t p) c -> p t c", t=3)
                src3 = Ktl[:, tlo * T:(tlo + 3) * T].rearrange(
                    "p (t c) -> p t c", t=3)
                kvdma.dma_start(out=kin3, in_=src3)
                for m in range(3):
                    rows = 128 if m < 2 else T - 256
                    kvdma.dma_start(
                        out=vin[m * 128:m * 128 + rows, :],
                        in_=Vnl[:rows, m * VCOLS + nlo: m * VCOLS + nlo + VH])

            def scatter_half(kv_out, tlo, nlo):
                for c in range(4):
                    kc3 = kv_out[c * KVH: c * KVH + KH_K].rearrange(
                        "(t p col) -> p t col", t=3, p=128)
                    dst3 = Ktf[:].rearrange("p (t k) -> p t k", k=KPAD)[
                        :, tlo:tlo + 3, c * T:(c + 1) * T]
                    kvdma.dma_start(out=dst3, in_=kc3)
                    # V rows for chunk c: tokens [c*T, (c+1)*T) across 128-row tiles
                    r = c * T
                    while r < (c + 1) * T:
                        m = r // 128
                        take = min((m + 1) * 128, (c + 1) * T) - r
                        vc = kv_out[c * KVH + KH_K + (r - c * T) * VH:
                                    c * KVH + KH_K + (r - c * T + take) * VH]
                        kvdma.dma_start(
                            out=Vnf[r - m * 128: r - m * 128 + take,
                                    m * VCOLS + nlo: m * VCOLS + nlo + VH],
                            in_=vc.rearrange("(a b) -> a b", b=VH))
                        r += take

            def attn_head(h):
                ft, row = h // 2, (h % 2) * 64
                qh = Qt[row:row + 64, ft * T:(ft + 1) * T]
                P = ppool.tile([128, KT * T], BF16, tag="P")

                def qk_group(m0, gn):
                    A = ps.tile([128, 1536], F32, tag="mm")
                    for j in range(gn):
                        m = m0 + j
                        kh = Ktf[row:row + 64, ft * KPAD + m * 128: ft * KPAD + (m + 1) * 128]
                        nc.tensor.matmul(A[:, j * 512:j * 512 + T], kh, qh,
                                         start=True, stop=True)
                    return A

                def exp_mask(A, m0, gn):
                    if NO_EXP3:
                        for j in range(gn):
                            nc.scalar.activation(
                                P[:, (m0 + j) * T:(m0 + j + 1) * T],
                                A[:, j * 512:j * 512 + T], AF.Exp, scale=float(SC))
                    else:
                        src = A[:].rearrange("p (a b) -> p a b", b=512)[:, 0:gn, 0:T]
                        dst = P[:, m0 * T:(m0 + gn) * T].rearrange("p (a b) -> p a b", b=T)
                        nc.scalar.activation(dst, src, AF.Exp, scale=float(SC))
                    nc.vector.tensor_mul(
                        P[:, m0 * T:(m0 + gn) * T], P[:, m0 * T:(m0 + gn) * T],
                        msk[:, (h * 8 + m0) * T:(h * 8 + m0 + gn) * T])

                def pv(m0, gn, start):
                    for m in range(m0, m0 + gn):
                        vh = Vnf[:, m * VCOLS + h * 65: m * VCOLS + (h + 1) * 65]
                        nc.tensor.matmul(cps[:], vh, P[:, m * T:(m + 1) * T],
                                         start=(m == m0 and start), stop=False)

                # interleave so only 2 QK psum slots are in flight at a time
                A0 = qk_group(0, 3)
                A1 = qk_group(3, 3)
                exp_mask(A0, 0, 3)
                # rank-1 tile for key 1024
                p8 = psc.tile([1, T], F32, tag="ctx")
                kh8 = Ktf[row:row + 64, ft * KPAD + 1024: ft * KPAD + 1025]
                nc.tensor.matmul(p8[:], kh8, qh, start=True, stop=True)
                nc.scalar.activation(P[0:1, 8 * T:9 * T], p8[0:1, :], AF.Exp,
                                     scale=float(SC))
                A2 = qk_group(6, 2)
                exp_mask(A1, 3, 3)
                cps = psc.tile([65, T], F32, tag="ctx")
                pv(0, 3, True)
                exp_mask(A2, 6, 2)
                pv(3, 3, False)
                pv(6, 2, False)
                nc.tensor.matmul(cps[:], Vnf[0:1, 8 * VCOLS + h * 65: 8 * VCOLS + (h + 1) * 65],
                                 P[0:1, 8 * T:9 * T], start=False, stop=True)
                # stash Z and unnormalized ctx; divide after all heads (batched)
                ztmp = work.tile([1, T], F32, tag="ztmp", bufs=6)
                nc.vector.tensor_copy(out=ztmp[:], in_=cps[64:65, :])
                nc.sync.dma_start(out=zall[h:h + 1, :], in_=ztmp[:])
                nc.vector.tensor_copy(out=ctx[row:row + 64, ft * T:(ft + 1) * T],
                                      in_=cps[0:64, :])

            # ---- layers ----------------------------------------------------
            for i in range(nlayers):
                lnp_sb = wp.tile([128, 72], F32, tag="lnp", bufs=2)
                bvb_sb = wp.tile([128, VCOLS], BF16, tag="bvb", bufs=2)
                nc.sync.dma_start(out=lnp_sb[:], in_=lnp[i][:, :])
                nc.sync.dma_start(out=bvb_sb[:], in_=bvb[i][:, :])

                # LN1
                layernorm(X, 0, xn)

                # K projection + V projection, split in head halves; AllGather
                # each half as soon as it is produced.
                kslabs = load_slabs(wk[i], "kslab", D)
                vslabs = load_slabs(wv[i], "vslab", VCOLS)

                def vproj_half(nlo):
                    for m in range(3):  # token tiles 128,128,16
                        rows = 128 if m < 2 else T - 256
                        psm = ps.tile([128, 1536], F32, tag="mm")
                        for t in range(DT):
                            nc.tensor.matmul(
                                psm[:rows, 0:VH],
                                xn[:, t * T + m * 128: t * T + m * 128 + rows],
                                vslabs[t][:, nlo:nlo + VH],
                                start=(t == 0), stop=(t == DT - 1))
                        nc.vector.tensor_add(
                            Vnl[:rows, m * VCOLS + nlo: m * VCOLS + nlo + VH],
                            psm[:rows, 0:VH], bvb_sb[:rows, nlo:nlo + VH])

                proj_out(kslabs, xn, Ktl, 30, range(0, 3))
                vproj_half(0)
                bounce_half(kinA, vinA, 0, 0)
                nc.gpsimd.collective_compute(
                    "AllGather", ALU.bypass, replica_groups=RG,
                    ins=[kva_in[:].opt()], outs=[kva_out[:].opt()])

                proj_out(kslabs, xn, Ktl, 30, range(3, 6))
                vproj_half(VH)

                # Q projection overlaps collective A
                qslabs = load_slabs(wq[i], "qslab", D)
                proj_out(qslabs, xn, Qt, 24, range(DT))

                scatter_half(kva_out, 0, 0)
                bounce_half(kinB, vinB, 3, VH)
                nc.gpsimd.collective_compute(
                    "AllGather", ALU.bypass, replica_groups=RG,
                    ins=[kvb_in[:].opt()], outs=[kvb_out[:].opt()])
                scatter_half(kvb_out, 3, VH)

                # ---- attention, head by head ----
                for h in range(NH):
                    attn_head(h)

                # batched 1/Z and per-head-pair normalization of ctx
                zr12 = stat.tile([NH, T], F32, tag="zr12")
                nc.vector.reciprocal(zr12[:], zall[:])
                zr12b = stat.tile([NH, T], BF16, tag="zr12b")
                nc.vector.tensor_copy(out=zr12b[:], in_=zr12[:])
                for f in range(DT):
                    zbps = ps.tile([128, 1536], F32, tag="mm")
                    nc.tensor.matmul(zbps[:, 0:T], esel_sb[:, f * 128:(f + 1) * 128],
                                     zr12b[:], start=True, stop=True)
                    nc.vector.tensor_mul(ctx[:, f * T:(f + 1) * T],
                                         ctx[:, f * T:(f + 1) * T], zbps[:, 0:T])

                # ---- Wo + residual (fused on DVE) ----
                oslabs = load_slabs(wo[i], "kslab", D)
                for o in range(DT):
                    psm = ps.tile([128, 1536], F32, tag="mm")
                    for t in range(DT):
                        nc.tensor.matmul(psm[:, 0:T], oslabs[t][:, o * 128:(o + 1) * 128],
                                         ctx[:, t * T:(t + 1) * T],
                                         start=(t == 0), stop=(t == DT - 1))
                    nc.vector.scalar_tensor_tensor(
                        out=X[:, o * T:(o + 1) * T], in0=psm[:, 0:T],
                        scalar=lnp_sb[:, 36 + o:37 + o], in1=X[:, o * T:(o + 1) * T],
                        op0=ALU.add, op1=ALU.add)

                # LN2 -> xn (reuse buffer)
                layernorm(X, 12, xn)

                # ---- FFN ----
                # w1 is host-tiled o-major: cols (o*DT + t)*128 hold W1-tile (t, o)
                for o in range(FT):
                    slab = wp.tile([128, 780], BF16, tag="vslab", bufs=6)
                    nc.sync.dma_start(out=slab[:, :D], in_=w1[i][:, o * D:(o + 1) * D])
                    psm = ps.tile([128, 1536], F32, tag="mm")
                    for t in range(DT):
                        nc.tensor.matmul(psm[:, 0:T], slab[:, t * 128:(t + 1) * 128],
                                         xn[:, t * T:(t + 1) * T],
                                         start=(t == 0), stop=(t == DT - 1))
                    nc.scalar.activation(hsb[:, o * T:(o + 1) * T], psm[:, 0:T], AF.Gelu,
                                         bias=lnp_sb[:, 48 + o:49 + o])
                # w2 host-tiled o-major: cols (o*FT + t)*128 hold W2-tile (t, o)
                for o in range(DT):
                    slab = wp.tile([128, F], BF16, tag="w2o", bufs=2)
                    nc.sync.dma_start(out=slab[:], in_=w2[i][:, o * F:(o + 1) * F])
                    psm = ps.tile([128, 1536], F32, tag="mm")
                    for t in range(FT):
                        nc.tensor.matmul(psm[:, 0:T], slab[:, t * 128:(t + 1) * 128],
                                         hsb[:, t * T:(t + 1) * T],
                                         start=(t == 0), stop=(t == FT - 1))
                    nc.vector.scalar_tensor_tensor(
                        out=X[:, o * T:(o + 1) * T], in0=psm[:, 0:T],
                        scalar=lnp_sb[:, 42 + o:43 + o], in1=X[:, o * T:(o + 1) * T],
                        op0=ALU.add, op1=ALU.add)

            # ---- final LN -> out (bf16, converted on host) ---------------
            lnp_sb = wp.tile([128, 72], F32, tag="lnp", bufs=2)
            nc.sync.dma_start(out=lnp_sb[:, 0:2 * DT], in_=normp_in[:, :])
            layernorm(X, 0, xn)
            for t in range(DT):
                nc.sync.dma_start(out=out_t[:, t * T:(t + 1) * T],
                                  in_=xn[:, t * T:(t + 1) * T])

    nc.compile()
    return nc


# ---------------------------------------------------------------- host prep

def _ft_pack(a):
    """[768, T] -> [128, 6*T] ft-tile-major."""
    Tn = a.shape[1]
    return a.reshape(DT, 128, Tn).transpose(1, 0, 2).reshape(128, DT * Tn)


def _pp_pack(v):
    """[n*128] per-feature -> [128, n] per-partition columns."""
    return np.ascontiguousarray(v.reshape(-1, 128).T)


def build_masks(rand_attn):
    """[NH, 8 k-tiles of 128 keys, T] per core r -> mask[r][128, NH*8*T]."""
    ra = np.asarray(rand_attn)
    # block-level MULTIPLICITY: cnt[h, l, j] = how many times k-block j appears
    # in the reference's concatenated key list for q-block l (duplicated random
    # blocks are counted twice in the reference softmax).
    cnt = np.zeros((NH, NBLK, NBLK), dtype=np.float32)
    cnt[:, 0, :] = 1.0
    cnt[:, 16, :] = 1.0
    for h in range(NH):
        for l in range(1, 16):
            base = {0, 16, l - 1, l, l + 1} if 1 < l < 15 else (
                {0, 1, 2, 16} if l == 1 else {0, 14, 15, 16})
            for j in base:
                cnt[h, l, j] += 1.0
            for r in range(R):
                cnt[h, l, int(ra[h, l - 1, r])] += 1.0
    masks = []
    for r in range(4):
        qg = np.arange(r * T, (r + 1) * T)
        lq = np.minimum(qg // BS, NBLK - 1)
        kg = np.arange(8 * 128)           # keys 0..1023, all real
        jk = kg // BS
        m = np.zeros((NH, 8 * 128, T), dtype=BF)
        for h in range(NH):
            m[h] = cnt[h].T[np.ix_(jk, lq)].astype(BF)
        m = m.reshape(NH, 8, 128, T).transpose(2, 0, 1, 3).reshape(128, NH * 8 * T)
        masks.append(np.ascontiguousarray(m))
    return masks


def prepare_inputs(inputs, nlayers=NLAYERS):
    pv = np.asarray(inputs["pixel_values"], np.float32)
    B = pv.shape[0]
    g_img = pv.shape[2] // 16
    ntok_img = g_img * g_img
    patches = pv.reshape(B, 3, g_img, 16, g_img, 16).transpose(0, 2, 4, 1, 3, 5)
    patches = patches.reshape(B, ntok_img, 768)

    pos = np.asarray(inputs["pos_emb"], np.float32)[0]          # [1025, 768]
    cls = np.asarray(inputs["cls_token"], np.float32).reshape(768)
    patch_b = np.asarray(inputs["patch_b"], np.float32)

    # patchesZ^T [768, 1088] and add_term [768, 1088] per batch
    pzt = np.zeros((B, 768, SEQP), np.float32)
    addt = np.zeros((B, 768, SEQP), np.float32)
    for b in range(B):
        pzt[b, :, 1:1 + ntok_img] = patches[b].T
        addt[b, :, 0] = cls + pos[0]
        addt[b, :, 1:SEQ] = (patch_b[None, :] + pos[1:SEQ]).T

    masks = build_masks(inputs["rand_attn"])

    def bfc(x):
        return np.ascontiguousarray(np.asarray(x, np.float32).astype(BF))

    esel = np.zeros((NH, DT * 128), np.float32)
    for f in range(DT):
        for p in range(128):
            esel[2 * f + (p >= 64), f * 128 + p] = 1.0
    shared = {"pw": bfc(inputs["patch_w"]), "esel": bfc(esel)}
    normp = np.concatenate(
        [_pp_pack(np.asarray(inputs["norm_g"], np.float32)),
         _pp_pack(np.asarray(inputs["norm_b"], np.float32))], axis=1)
    shared["normp"] = np.ascontiguousarray(normp)
    for i in range(nlayers):
        shared[f"wq{i}"] = bfc(inputs["Wq"][i])
        shared[f"wk{i}"] = bfc(inputs["Wk"][i])
        wva = np.zeros((768, VCOLS), np.float32)
        wv_i = np.asarray(inputs["Wv"][i], np.float32)
        for h in range(NH):
            wva[:, h * 65:h * 65 + 64] = wv_i[:, h * 64:(h + 1) * 64]
        shared[f"wv{i}"] = bfc(wva)
        shared[f"wo{i}"] = bfc(inputs["Wo"][i])
        # o-major tiling: [CT*128, OT*128] -> [128, OT*CT*128]
        w1_i = np.asarray(inputs["ff_w1"][i], np.float32)       # [768, 3072]
        w1t = w1_i.reshape(DT, 128, FT, 128).transpose(1, 2, 0, 3).reshape(128, FT * D)
        shared[f"w1{i}"] = bfc(w1t)
        w2_i = np.asarray(inputs["ff_w2"][i], np.float32)       # [3072, 768]
        w2t = w2_i.reshape(FT, 128, DT, 128).transpose(1, 2, 0, 3).reshape(128, DT * F)
        shared[f"w2{i}"] = bfc(w2t)
        lnp_i = np.zeros((128, 72), np.float32)
        lnp_i[:, 0:6] = _pp_pack(np.asarray(inputs["ln1_g"][i], np.float32))
        lnp_i[:, 6:12] = _pp_pack(np.asarray(inputs["ln1_b"][i], np.float32))
        lnp_i[:, 12:18] = _pp_pack(np.asarray(inputs["ln2_g"][i], np.float32))
        lnp_i[:, 18:24] = _pp_pack(np.asarray(inputs["ln2_b"][i], np.float32))
        lnp_i[:, 24:30] = _pp_pack(np.asarray(inputs["bq"][i], np.float32))
        lnp_i[:, 30:36] = _pp_pack(np.asarray(inputs["bk"][i], np.float32))
        lnp_i[:, 36:42] = _pp_pack(np.asarray(inputs["bo"][i], np.float32))
        lnp_i[:, 42:48] = _pp_pack(np.asarray(inputs["ff_b2"][i], np.float32))
        lnp_i[:, 48:72] = _pp_pack(np.asarray(inputs["ff_b1"][i], np.float32))
        shared[f"lnp{i}"] = np.ascontiguousarray(lnp_i)
        bva = np.zeros((VCOLS,), np.float32)
        bv_i = np.asarray(inputs["bv"][i], np.float32)
        for h in range(NH):
            bva[h * 65:h * 65 + 64] = bv_i[h * 64:(h + 1) * 64]
            bva[h * 65 + 64] = 1.0
        shared[f"bvb{i}"] = np.ascontiguousarray(
            np.broadcast_to(bva.astype(BF), (128, VCOLS)))

    in_maps = []
    for c in range(8):
        g, r = c // 4, c % 4
        im = dict(shared)
        sl = slice(r * T, (r + 1) * T)
        im["pe_in"] = np.ascontiguousarray(_ft_pack(pzt[g][:, sl]).astype(BF))
        im["add_in"] = np.ascontiguousarray(_ft_pack(addt[g][:, sl]))
        im["mask_in"] = masks[r]
        in_maps.append(im)
    return in_maps


LAST_RESULT = None


def kernel(**inputs):
    global LAST_RESULT
    key = ("prog", NLAYERS)
    if key not in _CACHE:
        _CACHE[key] = build_program(NLAYERS)
    nc = _CACHE[key]
    in_maps = prepare_inputs(inputs, NLAYERS)
    kw = {}
    if os.environ.get("BB_TRACE"):
        kw = dict(trace=True, trace_cores=[0])
    res = run_bass_kernel_spmd(nc, in_maps, core_ids=list(range(8)), **kw)
    LAST_RESULT = res
    outs = []
    for g in range(2):
        cols = []
        for r in range(4):
            o = res.results[g * 4 + r]["out"]          # [128, 6*T]
            o = o.reshape(128, DT, T).transpose(1, 0, 2).reshape(768, T)
            cols.append(o)
        xt = np.concatenate(cols, axis=1)              # [768, 1088]
        outs.append(xt[:, :SEQ].T)                     # [1025, 768]
    return np.stack(outs, axis=0).astype(np.float32)


if __name__ == "__main__":
    import reference
    ins = {k: np.asarray(v) for k, v in reference.setup_inputs().items()}
    got = kernel(**ins)
    print("kernel output", got.shape)


# revision 20
# speedup vs baseline: 1.0033x; 1.0033x over previous
"""BigBird ViT forward on 8 Trainium2 NeuronCores.

Sharding: 2 groups of 4 cores (one per batch element). Within a group,
tokens are sharded 4-way (272 of the 1088 padded tokens per core) for all
dense matmuls / layernorms (weights replicated, streamed from HBM in bf16),
and attention is computed for the core's own 272 query tokens over all 12
heads, after a per-layer AllGather of K^T and V split in two halves by
head group (heads 0-5 / 6-11) so the second gather overlaps attention on
the first half.

Everything on-chip lives transposed ([feature, token]) so the PE contracts
over partitions without any activation transposes; LayerNorm reductions over
the feature dim use ones-vector matmuls in bf16 on the PE.

The BigBird band/random/global structure (plus seq padding) is applied as a
multiplicative {0,1,2} bf16 mask on the unnormalized attention probabilities;
with S=1025 the reference's -10000 additive masking underflows exp() to
exactly 0, so this is an exact reformulation. Keys are truncated to the real
1025 (k-tiles 0-7 = keys 0..1023 plus a rank-1 tile for key 1024).
"""
import os
import sys

sys.path.insert(0, "/opt/trn_rl_repo")

import numpy as np
import ml_dtypes

import concourse.bass as bass
import concourse.bacc as bacc
import concourse.mybir as mybir
import concourse.tile as tile
from concourse.bass_utils import run_bass_kernel_spmd

F32 = mybir.dt.float32
BF16 = mybir.dt.bfloat16
AF = mybir.ActivationFunctionType
ALU = mybir.AluOpType
BF = ml_dtypes.bfloat16

# model dims
BS = 64; NH = 12; HD = 64; D = 768; F = 3072; L = 12; R = 3
SEQ = 1025
SEQP = 1088           # padded to 17 blocks of 64
NBLK = 17
T = SEQP // 4         # tokens per core = 272
DT = D // 128         # 6 feature tiles
FT = F // 128         # 24 ffn tiles
KT = 9                # conceptual k tiles; tiles 0-7 full, tile 8 = key 1024
KPAD = 1152           # Ktf column stride per feature tile
VCOLS = NH * (HD + 1)  # 780: per-head [64 V cols + 1 ones col]
VH = VCOLS // 2        # 390 (heads 0-5 | 6-11)
SC = 1.0 / np.sqrt(HD)

NLAYERS = int(os.environ.get("BB_NLAYERS", str(L)))
# partition_broadcast crashes and reciprocal_approx_fast mis-computes on this
# runtime (missing gpsimd / custom-DVE ucode); use PE broadcast + exact recip.
NO_EXP3 = bool(os.environ.get("BB_NO_EXP3"))   # avoid 3D-AP batched exp
NO_GPDMA = bool(os.environ.get("BB_NO_GPDMA")) # avoid gpsimd-issued DMAs

_CACHE = {}

RG = [[0, 1, 2, 3], [4, 5, 6, 7]]


# ---------------------------------------------------------------- builder

def build_program(nlayers=NLAYERS):
    nc = bacc.Bacc("TRN2", target_bir_lowering=False, debug=False, num_devices=8)

    # ---- DRAM I/O -------------------------------------------------------
    pe_in = nc.dram_tensor("pe_in", [128, DT * T], BF16, kind="ExternalInput")
    add_in = nc.dram_tensor("add_in", [128, DT * T], F32, kind="ExternalInput")
    mask_in = nc.dram_tensor("mask_in", [128, NH * 8 * T], BF16, kind="ExternalInput")
    pw_in = nc.dram_tensor("pw", [D, D], BF16, kind="ExternalInput")
    normp_in = nc.dram_tensor("normp", [128, 2 * DT], F32, kind="ExternalInput")
    wq = [nc.dram_tensor(f"wq{i}", [D, D], BF16, kind="ExternalInput") for i in range(nlayers)]
    wk = [nc.dram_tensor(f"wk{i}", [D, D], BF16, kind="ExternalInput") for i in range(nlayers)]
    wv = [nc.dram_tensor(f"wv{i}", [D, VCOLS], BF16, kind="ExternalInput") for i in range(nlayers)]
    wo = [nc.dram_tensor(f"wo{i}", [D, D], BF16, kind="ExternalInput") for i in range(nlayers)]
    # w1/w2 shipped pre-tiled o-major: [128, OT*CT*128] with each 128x128 tile
    # contiguous, all contraction tiles of one output tile adjacent.
    w1 = [nc.dram_tensor(f"w1{i}", [128, FT * D], BF16, kind="ExternalInput") for i in range(nlayers)]
    w2 = [nc.dram_tensor(f"w2{i}", [128, DT * F], BF16, kind="ExternalInput") for i in range(nlayers)]
    lnp = [nc.dram_tensor(f"lnp{i}", [128, 72], F32, kind="ExternalInput") for i in range(nlayers)]
    bvb = [nc.dram_tensor(f"bvb{i}", [128, VCOLS], BF16, kind="ExternalInput") for i in range(nlayers)]
    esel_in = nc.dram_tensor("esel", [NH, DT * 128], BF16, kind="ExternalInput")
    out_t = nc.dram_tensor("out", [128, DT * T], BF16, kind="ExternalOutput")

    # collective bounce buffers (internal DRAM), reused across layers.
    # Each half: K^T for 3 feature tiles + V cols for 6 heads.
    KH_K = 3 * 128 * T
    KH_V = T * VH
    KVH = KH_K + KH_V
    kva_in = nc.dram_tensor("kva_in", [KVH], BF16)
    kvb_in = nc.dram_tensor("kvb_in", [KVH], BF16)
    kva_out = nc.dram_tensor("kva_out", [4 * KVH], BF16)
    kvb_out = nc.dram_tensor("kvb_out", [4 * KVH], BF16)

    with tile.TileContext(nc) as tc:
        # ---- persistent SBUF tensors -----------------------------------
        X = nc.alloc_sbuf_tensor("X", [128, DT * T], F32)          # residual, ft-tile major
        Xbf = nc.alloc_sbuf_tensor("Xbf", [128, DT * T], BF16)     # bf16 copy for LN stats
        xn = nc.alloc_sbuf_tensor("xn", [128, DT * T], BF16)       # LN output
        Qt = nc.alloc_sbuf_tensor("Qt", [128, DT * T], BF16)       # Q^T local
        Ktl = nc.alloc_sbuf_tensor("Ktl", [128, DT * T], BF16)     # K^T local
        Vnl = nc.alloc_sbuf_tensor("Vnl", [128, 3 * VCOLS], BF16)  # V-nat local (3 tok tiles)
        Ktf = nc.alloc_sbuf_tensor("Ktf", [128, DT * KPAD], BF16)  # K^T full
        Vnf = nc.alloc_sbuf_tensor("Vnf", [128, KT * VCOLS], BF16)  # V-nat full
        ctx = nc.alloc_sbuf_tensor("ctx", [128, DT * T], BF16)     # attention out^T
        hsb = nc.alloc_sbuf_tensor("hsb", [128, FT * T], BF16)     # ffn hidden^T
        msk = nc.alloc_sbuf_tensor("msk", [128, NH * 8 * T], BF16)
        onesb = nc.alloc_sbuf_tensor("onesb", [128, 1], BF16)      # for LN stat matmuls
        ones1 = nc.alloc_sbuf_tensor("ones1", [1, 128], BF16)      # fallback PE bcast
        ones1f = nc.alloc_sbuf_tensor("ones1f", [1, 128], F32)
        mu_b = nc.alloc_sbuf_tensor("mu_b", [128, T], BF16)
        rs_b = nc.alloc_sbuf_tensor("rs_b", [128, T], BF16)
        epsb = nc.alloc_sbuf_tensor("epsb", [128, 1], F32)
        zall = nc.alloc_sbuf_tensor("zall", [NH, T], F32)          # per-head softmax Z
        esel_sb = nc.alloc_sbuf_tensor("esel_sb", [NH, DT * 128], BF16)

        with (
            tc.tile_pool(name="wp", bufs=1) as wp,              # weight slabs (tags w/ bufs)
            tc.tile_pool(name="work", bufs=2) as work,
            tc.tile_pool(name="stat", bufs=2) as stat,
            tc.tile_pool(name="ppool", bufs=3) as ppool,        # P tiles per head
            tc.tile_pool(name="ps", bufs=2, space="PSUM") as ps,       # [128,1536] slots
            tc.tile_pool(name="psc", bufs=2, space="PSUM") as psc,     # PV accum + p8 + LN stats
        ):
            nc.vector.memset(onesb[:], 1.0)
            nc.vector.memset(ones1[:], 1.0)
            nc.vector.memset(ones1f[:], 1.0)
            nc.vector.memset(epsb[:], 1e-5)

            kvdma = nc.sync if NO_GPDMA else nc.gpsimd

            def bcast(dst, src_row):
                """broadcast [1,T] src_row to dst [P,T] via K=1 PE matmul."""
                p = dst.partition_size()
                one = ones1f if src_row.dtype == F32 else ones1
                bps = ps.tile([128, 1536], F32, tag="mm")
                nc.tensor.matmul(bps[:p, 0:T], one[:, 0:p], src_row,
                                 start=True, stop=True)
                nc.vector.tensor_copy(out=dst, in_=bps[:p, 0:T])
            # load masks + Z selector (resident)
            nc.sync.dma_start(out=msk[:], in_=mask_in[:, :])
            nc.sync.dma_start(out=esel_sb[:], in_=esel_in[:, :])

            def layernorm(src_f32, gcol, out_bf):
                """src [128, DT*T] f32 ft-major -> out bf16, LN over features."""
                # bf16 copy for cheap stat matmuls (1 cycle/row vs 4 for f32)
                for t in range(DT):
                    nc.vector.tensor_copy(
                        out=Xbf[:, t * T:(t + 1) * T], in_=src_f32[:, t * T:(t + 1) * T])
                sum_ps = psc.tile([1, T], F32, tag="ctx")
                for t in range(DT):
                    nc.tensor.matmul(sum_ps[:], onesb[:], Xbf[:, t * T:(t + 1) * T],
                                     start=(t == 0), stop=(t == DT - 1))
                mu = stat.tile([1, T], BF16, tag="mu")
                nc.scalar.activation(mu[:], sum_ps[:], AF.Identity, scale=1.0 / D)
                sq_ps = psc.tile([1, T], F32, tag="ctx")
                for t in range(DT):
                    sq = work.tile([128, T], BF16, tag="lnsq")
                    nc.vector.tensor_mul(sq[:], Xbf[:, t * T:(t + 1) * T],
                                         Xbf[:, t * T:(t + 1) * T])
                    nc.tensor.matmul(sq_ps[:], onesb[:], sq[:],
                                     start=(t == 0), stop=(t == DT - 1))
                var = stat.tile([1, T], F32, tag="var")
                nc.scalar.activation(var[:], sq_ps[:], AF.Identity, scale=1.0 / D)
                mu2 = stat.tile([1, T], F32, tag="mu2")
                nc.vector.tensor_mul(mu2[:], mu[:], mu[:])
                nc.vector.tensor_sub(var[:], var[:], mu2[:])
                std = stat.tile([1, T], F32, tag="std")
                nc.scalar.activation(std[:], var[:], AF.Sqrt, bias=epsb[0:1, 0:1])
                rstd32 = stat.tile([1, T], F32, tag="rstd32")
                nc.vector.reciprocal(rstd32[:], std[:])
                rstd = stat.tile([1, T], BF16, tag="rstd")
                nc.vector.tensor_copy(out=rstd[:], in_=rstd32[:])
                # broadcast mu, rstd to [128, T]
                bcast(mu_b[:], mu[0:1, :])
                bcast(rs_b[:], rstd[0:1, :])
                g = lnp_sb[:, gcol:gcol + DT]
                b = lnp_sb[:, gcol + DT:gcol + 2 * DT]
                for t in range(DT):
                    tmp = work.tile([128, T], BF16, tag="lnt")
                    nc.vector.tensor_sub(tmp[:], src_f32[:, t * T:(t + 1) * T], mu_b[:])
                    nc.vector.tensor_mul(tmp[:], tmp[:], rs_b[:])
                    nc.vector.tensor_scalar(
                        out_bf[:, t * T:(t + 1) * T], tmp[:],
                        g[:, t:t + 1], b[:, t:t + 1], op0=ALU.mult, op1=ALU.add)

            def load_slabs(wdram, tag, ncols):
                slabs = []
                for t in range(DT):
                    slab = wp.tile([128, 780], BF16, tag=tag, bufs=6)
                    nc.sync.dma_start(out=slab[:, :ncols],
                                      in_=wdram[t * 128:(t + 1) * 128, :])
                    slabs.append(slab)
                return slabs

            def proj_out(slabs, src_bf, out_bf, bias_col, orange):
                """out^T tiles (o in orange) = w^T @ src + bias, evac on DVE."""
                for o in orange:
                    psm = ps.tile([128, 1536], F32, tag="mm")
                    for t in range(DT):
                        nc.tensor.matmul(psm[:, 0:T], slabs[t][:, o * 128:(o + 1) * 128],
                                         src_bf[:, t * T:(t + 1) * T],
                                         start=(t == 0), stop=(t == DT - 1))
                    nc.vector.tensor_scalar(
                        out_bf[:, o * T:(o + 1) * T], psm[:, 0:T],
                        lnp_sb[:, bias_col + o:bias_col + o + 1], None, op0=ALU.add)

            # ---- embedding -------------------------------------------------
            eslabs = load_slabs(pw_in, "qslab", D)
            for o in range(DT):
                psm = ps.tile([128, 1536], F32, tag="mm")
                for t in range(DT):
                    peint = work.tile([128, T], BF16, tag="peint")
                    nc.sync.dma_start(out=peint[:], in_=pe_in[:, t * T:(t + 1) * T])
                    nc.tensor.matmul(psm[:, 0:T], eslabs[t][:, o * 128:(o + 1) * 128],
                                     peint[:], start=(t == 0), stop=(t == DT - 1))
                addt = work.tile([128, T], F32, tag="addt")
                nc.sync.dma_start(out=addt[:], in_=add_in[:, o * T:(o + 1) * T])
                nc.vector.tensor_add(X[:, o * T:(o + 1) * T], psm[:, 0:T], addt[:])

            kinA = kva_in[:KH_K].rearrange("(a b) -> a b", b=T)       # [384, T]
            vinA = kva_in[KH_K:].rearrange("(a b) -> a b", b=VH)      # [T, 390]
            kinB = kvb_in[:KH_K].rearrange("(a b) -> a b", b=T)
            vinB = kvb_in[KH_K:].rearrange("(a b) -> a b", b=VH)

            def bounce_half(kin, vin, tlo, nlo):
                kin3 = kin.rearrange("(t p) c -> p t c", t=3)
                src3 = Ktl[:, tlo * T:(tlo + 3) * T].rearrange(
                    "p (t c) -> p t c", t=3)
                kvdma.dma_start(out=kin3, in_=src3)
                for m in range(3):
                    rows = 128 if m < 2 else T - 256
                    kvdma.dma_start(
                        out=vin[m * 128:m * 128 + rows, :],
                        in_=Vnl[:rows, m * VCOLS + nlo: m * VCOLS + nlo + VH])

            def scatter_half(kv_out, tlo, nlo):
                for c in range(4):
                    kc3 = kv_out[c * KVH: c * KVH + KH_K].rearrange(
                        "(t p col) -> p t col", t=3, p=128)
                    dst3 = Ktf[:].rearrange("p (t k) -> p t k", k=KPAD)[
                        :, tlo:tlo + 3, c * T:(c + 1) * T]
                    kvdma.dma_start(out=dst3, in_=kc3)
                    # V rows for chunk c: tokens [c*T, (c+1)*T) across 128-row tiles
                    r = c * T
                    while r < (c + 1) * T:
                        m = r // 128
                        take = min((m + 1) * 128, (c + 1) * T) - r
                        vc = kv_out[c * KVH + KH_K + (r - c * T) * VH:
                                    c * KVH + KH_K + (r - c * T + take) * VH]
                        kvdma.dma_start(
                            out=Vnf[r - m * 128: r - m * 128 + take,
                                    m * VCOLS + nlo: m * VCOLS + nlo + VH],
                            in_=vc.rearrange("(a b) -> a b", b=VH))
                        r += take

            def attn_head(h):
                ft, row = h // 2, (h % 2) * 64
                qh = Qt[row:row + 64, ft * T:(ft + 1) * T]
                P = ppool.tile([128, KT * T], BF16, tag="P")

                def qk_group(m0, gn):
                    A = ps.tile([128, 1536], F32, tag="mm")
                    for j in range(gn):
                        m = m0 + j
                        kh = Ktf[row:row + 64, ft * KPAD + m * 128: ft * KPAD + (m + 1) * 128]
                        nc.tensor.matmul(A[:, j * 512:j * 512 + T], kh, qh,
                                         start=True, stop=True)
                    return A

                def exp_mask(A, m0, gn):
                    if NO_EXP3:
                        for j in range(gn):
                            nc.scalar.activation(
                                P[:, (m0 + j) * T:(m0 + j + 1) * T],
                                A[:, j * 512:j * 512 + T], AF.Exp, scale=float(SC))
                    else:
                        src = A[:].rearrange("p (a b) -> p a b", b=512)[:, 0:gn, 0:T]
                        dst = P[:, m0 * T:(m0 + gn) * T].rearrange("p (a b) -> p a b", b=T)
                        nc.scalar.activation(dst, src, AF.Exp, scale=float(SC))
                    nc.vector.tensor_mul(
                        P[:, m0 * T:(m0 + gn) * T], P[:, m0 * T:(m0 + gn) * T],
                        msk[:, (h * 8 + m0) * T:(h * 8 + m0 + gn) * T])

                def pv(m0, gn, start):
                    for m in range(m0, m0 + gn):
                        vh = Vnf[:, m * VCOLS + h * 65: m * VCOLS + (h + 1) * 65]
                        nc.tensor.matmul(cps[:], vh, P[:, m * T:(m + 1) * T],
                                         start=(m == m0 and start), stop=False)

                # interleave so only 2 QK psum slots are in flight at a time
                A0 = qk_group(0, 3)
                A1 = qk_group(3, 3)
                exp_mask(A0, 0, 3)
                # rank-1 tile for key 1024
                p8 = psc.tile([1, T], F32, tag="ctx")
                kh8 = Ktf[row:row + 64, ft * KPAD + 1024: ft * KPAD + 1025]
                nc.tensor.matmul(p8[:], kh8, qh, start=True, stop=True)
                nc.scalar.activation(P[0:1, 8 * T:9 * T], p8[0:1, :], AF.Exp,
                                     scale=float(SC))
                A2 = qk_group(6, 2)
                exp_mask(A1, 3, 3)
                cps = psc.tile([65, T], F32, tag="ctx")
                pv(0, 3, True)
                exp_mask(A2, 6, 2)
                pv(3, 3, False)
                pv(6, 2, False)
                nc.tensor.matmul(cps[:], Vnf[0:1, 8 * VCOLS + h * 65: 8 * VCOLS + (h + 1) * 65],
                                 P[0:1, 8 * T:9 * T], start=False, stop=True)
                # stash Z and unnormalized ctx; divide after all heads (batched)
                ztmp = work.tile([1, T], F32, tag="ztmp", bufs=3)
                nc.vector.tensor_copy(out=ztmp[:], in_=cps[64:65, :])
                nc.sync.dma_start(out=zall[h:h + 1, :], in_=ztmp[:])
                nc.vector.tensor_copy(out=ctx[row:row + 64, ft * T:(ft + 1) * T],
                                      in_=cps[0:64, :])

            # ---- layers ----------------------------------------------------
            for i in range(nlayers):
                lnp_sb = wp.tile([128, 72], F32, tag="lnp", bufs=2)
                bvb_sb = wp.tile([128, VCOLS], BF16, tag="bvb", bufs=2)
                nc.sync.dma_start(out=lnp_sb[:], in_=lnp[i][:, :])
                nc.sync.dma_start(out=bvb_sb[:], in_=bvb[i][:, :])

                # LN1
                layernorm(X, 0, xn)

                # K projection + V projection, split in head halves; AllGather
                # each half as soon as it is produced.
                kslabs = load_slabs(wk[i], "kslab", D)
                vslabs = load_slabs(wv[i], "vslab", VCOLS)

                def vproj_half(nlo):
                    for m in range(3):  # token tiles 128,128,16
                        rows = 128 if m < 2 else T - 256
                        psm = ps.tile([128, 1536], F32, tag="mm")
                        for t in range(DT):
                            nc.tensor.matmul(
                                psm[:rows, 0:VH],
                                xn[:, t * T + m * 128: t * T + m * 128 + rows],
                                vslabs[t][:, nlo:nlo + VH],
                                start=(t == 0), stop=(t == DT - 1))
                        nc.vector.tensor_add(
                            Vnl[:rows, m * VCOLS + nlo: m * VCOLS + nlo + VH],
                            psm[:rows, 0:VH], bvb_sb[:rows, nlo:nlo + VH])

                proj_out(kslabs, xn, Ktl, 30, range(0, 3))
                vproj_half(0)
                bounce_half(kinA, vinA, 0, 0)
                nc.gpsimd.collective_compute(
                    "AllGather", ALU.bypass, replica_groups=RG,
                    ins=[kva_in[:].opt()], outs=[kva_out[:].opt()])

                proj_out(kslabs, xn, Ktl, 30, range(3, 6))
                vproj_half(VH)

                # Q projection overlaps collective A
                qslabs = load_slabs(wq[i], "qslab", D)
                proj_out(qslabs, xn, Qt, 24, range(DT))

                scatter_half(kva_out, 0, 0)
                bounce_half(kinB, vinB, 3, VH)
                nc.gpsimd.collective_compute(
                    "AllGather", ALU.bypass, replica_groups=RG,
                    ins=[kvb_in[:].opt()], outs=[kvb_out[:].opt()])
                scatter_half(kvb_out, 3, VH)

                # ---- attention, head by head ----
                for h in range(NH):
                    attn_head(h)

                # batched 1/Z and per-head-pair normalization of ctx
                zr12 = stat.tile([NH, T], F32, tag="zr12")
                nc.vector.reciprocal(zr12[:], zall[:])
                zr12b = stat.tile([NH, T], BF16, tag="zr12b")
                nc.vector.tensor_copy(out=zr12b[:], in_=zr12[:])
                for f in range(DT):
                    zbps = ps.tile([128, 1536], F32, tag="mm")
                    nc.tensor.matmul(zbps[:, 0:T], esel_sb[:, f * 128:(f + 1) * 128],
                                     zr12b[:], start=True, stop=True)
                    nc.vector.tensor_mul(ctx[:, f * T:(f + 1) * T],
                                         ctx[:, f * T:(f + 1) * T], zbps[:, 0:T])

                # ---- Wo + residual (fused on DVE) ----
                oslabs = load_slabs(wo[i], "kslab", D)
                for o in range(DT):
                    psm = ps.tile([128, 1536], F32, tag="mm")
                    for t in range(DT):
                        nc.tensor.matmul(psm[:, 0:T], oslabs[t][:, o * 128:(o + 1) * 128],
                                         ctx[:, t * T:(t + 1) * T],
                                         start=(t == 0), stop=(t == DT - 1))
                    nc.vector.scalar_tensor_tensor(
                        out=X[:, o * T:(o + 1) * T], in0=psm[:, 0:T],
                        scalar=lnp_sb[:, 36 + o:37 + o], in1=X[:, o * T:(o + 1) * T],
                        op0=ALU.add, op1=ALU.add)

                # LN2 -> xn (reuse buffer)
                layernorm(X, 12, xn)

                # ---- FFN ----
                # w1 is host-tiled o-major: cols (o*DT + t)*128 hold W1-tile (t, o)
                for o in range(FT):
                    slab = wp.tile([128, 780], BF16, tag="vslab", bufs=6)
                    nc.sync.dma_start(out=slab[:, :D], in_=w1[i][:, o * D:(o + 1) * D])
                    psm = ps.tile([128, 1536], F32, tag="mm")
                    for t in range(DT):
                        nc.tensor.matmul(psm[:, 0:T], slab[:, t * 128:(t + 1) * 128],
                                         xn[:, t * T:(t + 1) * T],
                                         start=(t == 0), stop=(t == DT - 1))
                    nc.scalar.activation(hsb[:, o * T:(o + 1) * T], psm[:, 0:T], AF.Gelu,
                                         bias=lnp_sb[:, 48 + o:49 + o])
                # w2 host-tiled o-major: cols (o*FT + t)*128 hold W2-tile (t, o)
                for o in range(DT):
                    slab = wp.tile([128, F], BF16, tag="w2o", bufs=2)
                    nc.sync.dma_start(out=slab[:], in_=w2[i][:, o * F:(o + 1) * F])
                    psm = ps.tile([128, 1536], F32, tag="mm")
                    for t in range(FT):
                        nc.tensor.matmul(psm[:, 0:T], slab[:, t * 128:(t + 1) * 128],
                                         hsb[:, t * T:(t + 1) * T],
                                         start=(t == 0), stop=(t == FT - 1))
                    nc.vector.scalar_tensor_tensor(
                        out=X[:, o * T:(o + 1) * T], in0=psm[:, 0:T],
                        scalar=lnp_sb[:, 42 + o:43 + o], in1=X[:, o * T:(o + 1) * T],
                        op0=ALU.add, op1=ALU.add)

            # ---- final LN -> out (bf16, converted on host) ---------------
            lnp_sb = wp.tile([128, 72], F32, tag="lnp", bufs=2)
            nc.sync.dma_start(out=lnp_sb[:, 0:2 * DT], in_=normp_in[:, :])
            layernorm(X, 0, xn)
            for t in range(DT):
                nc.sync.dma_start(out=out_t[:, t * T:(t + 1) * T],
                                  in_=xn[:, t * T:(t + 1) * T])

    nc.compile()
    return nc


# ---------------------------------------------------------------- host prep

def _ft_pack(a):
    """[768, T] -> [128, 6*T] ft-tile-major."""
    Tn = a.shape[1]
    return a.reshape(DT, 128, Tn).transpose(1, 0, 2).reshape(128, DT * Tn)


def _pp_pack(v):
    """[n*128] per-feature -> [128, n] per-partition columns."""
    return np.ascontiguousarray(v.reshape(-1, 128).T)


def build_masks(rand_attn):
    """[NH, 8 k-tiles of 128 keys, T] per core r -> mask[r][128, NH*8*T]."""
    ra = np.asarray(rand_attn)
    # block-level MULTIPLICITY: cnt[h, l, j] = how many times k-block j appears
    # in the reference's concatenated key list for q-block l (duplicated random
    # blocks are counted twice in the reference softmax).
    cnt = np.zeros((NH, NBLK, NBLK), dtype=np.float32)
    cnt[:, 0, :] = 1.0
    cnt[:, 16, :] = 1.0
    for h in range(NH):
        for l in range(1, 16):
            base = {0, 16, l - 1, l, l + 1} if 1 < l < 15 else (
                {0, 1, 2, 16} if l == 1 else {0, 14, 15, 16})
            for j in base:
                cnt[h, l, j] += 1.0
            for r in range(R):
                cnt[h, l, int(ra[h, l - 1, r])] += 1.0
    masks = []
    for r in range(4):
        qg = np.arange(r * T, (r + 1) * T)
        lq = np.minimum(qg // BS, NBLK - 1)
        kg = np.arange(8 * 128)           # keys 0..1023, all real
        jk = kg // BS
        m = np.zeros((NH, 8 * 128, T), dtype=BF)
        for h in range(NH):
            m[h] = cnt[h].T[np.ix_(jk, lq)].astype(BF)
        m = m.reshape(NH, 8, 128, T).transpose(2, 0, 1, 3).reshape(128, NH * 8 * T)
        masks.append(np.ascontiguousarray(m))
    return masks


def prepare_inputs(inputs, nlayers=NLAYERS):
    pv = np.asarray(inputs["pixel_values"], np.float32)
    B = pv.shape[0]
    g_img = pv.shape[2] // 16
    ntok_img = g_img * g_img
    patches = pv.reshape(B, 3, g_img, 16, g_img, 16).transpose(0, 2, 4, 1, 3, 5)
    patches = patches.reshape(B, ntok_img, 768)

    pos = np.asarray(inputs["pos_emb"], np.float32)[0]          # [1025, 768]
    cls = np.asarray(inputs["cls_token"], np.float32).reshape(768)
    patch_b = np.asarray(inputs["patch_b"], np.float32)

    # patchesZ^T [768, 1088] and add_term [768, 1088] per batch
    pzt = np.zeros((B, 768, SEQP), np.float32)
    addt = np.zeros((B, 768, SEQP), np.float32)
    for b in range(B):
        pzt[b, :, 1:1 + ntok_img] = patches[b].T
        addt[b, :, 0] = cls + pos[0]
        addt[b, :, 1:SEQ] = (patch_b[None, :] + pos[1:SEQ]).T

    masks = build_masks(inputs["rand_attn"])

    def bfc(x):
        return np.ascontiguousarray(np.asarray(x, np.float32).astype(BF))

    esel = np.zeros((NH, DT * 128), np.float32)
    for f in range(DT):
        for p in range(128):
            esel[2 * f + (p >= 64), f * 128 + p] = 1.0
    shared = {"pw": bfc(inputs["patch_w"]), "esel": bfc(esel)}
    normp = np.concatenate(
        [_pp_pack(np.asarray(inputs["norm_g"], np.float32)),
         _pp_pack(np.asarray(inputs["norm_b"], np.float32))], axis=1)
    shared["normp"] = np.ascontiguousarray(normp)
    for i in range(nlayers):
        shared[f"wq{i}"] = bfc(inputs["Wq"][i])
        shared[f"wk{i}"] = bfc(inputs["Wk"][i])
        wva = np.zeros((768, VCOLS), np.float32)
        wv_i = np.asarray(inputs["Wv"][i], np.float32)
        for h in range(NH):
            wva[:, h * 65:h * 65 + 64] = wv_i[:, h * 64:(h + 1) * 64]
        shared[f"wv{i}"] = bfc(wva)
        shared[f"wo{i}"] = bfc(inputs["Wo"][i])
        # o-major tiling: [CT*128, OT*128] -> [128, OT*CT*128]
        w1_i = np.asarray(inputs["ff_w1"][i], np.float32)       # [768, 3072]
        w1t = w1_i.reshape(DT, 128, FT, 128).transpose(1, 2, 0, 3).reshape(128, FT * D)
        shared[f"w1{i}"] = bfc(w1t)
        w2_i = np.asarray(inputs["ff_w2"][i], np.float32)       # [3072, 768]
        w2t = w2_i.reshape(FT, 128, DT, 128).transpose(1, 2, 0, 3).reshape(128, DT * F)
        shared[f"w2{i}"] = bfc(w2t)
        lnp_i = np.zeros((128, 72), np.float32)
        lnp_i[:, 0:6] = _pp_pack(np.asarray(inputs["ln1_g"][i], np.float32))
        lnp_i[:, 6:12] = _pp_pack(np.asarray(inputs["ln1_b"][i], np.float32))
        lnp_i[:, 12:18] = _pp_pack(np.asarray(inputs["ln2_g"][i], np.float32))
        lnp_i[:, 18:24] = _pp_pack(np.asarray(inputs["ln2_b"][i], np.float32))
        lnp_i[:, 24:30] = _pp_pack(np.asarray(inputs["bq"][i], np.float32))
        lnp_i[:, 30:36] = _pp_pack(np.asarray(inputs["bk"][i], np.float32))
        lnp_i[:, 36:42] = _pp_pack(np.asarray(inputs["bo"][i], np.float32))
        lnp_i[:, 42:48] = _pp_pack(np.asarray(inputs["ff_b2"][i], np.float32))
        lnp_i[:, 48:72] = _pp_pack(np.asarray(inputs["ff_b1"][i], np.float32))
        shared[f"lnp{i}"] = np.ascontiguousarray(lnp_i)
        bva = np.zeros((VCOLS,), np.float32)
        bv_i = np.asarray(inputs["bv"][i], np.float32)
        for h in range(NH):
            bva[h * 65:h * 65 + 64] = bv_i[h * 64:(h + 1) * 64]
            bva[h * 65 + 64] = 1.0
        shared[f"bvb{i}"] = np.ascontiguousarray(
            np.broadcast_to(bva.astype(BF), (128, VCOLS)))

    in_maps = []
    for c in range(8):
        g, r = c // 4, c % 4
        im = dict(shared)
        sl = slice(r * T, (r + 1) * T)
        im["pe_in"] = np.ascontiguousarray(_ft_pack(pzt[g][:, sl]).astype(BF))
        im["add_in"] = np.ascontiguousarray(_ft_pack(addt[g][:, sl]))
        im["mask_in"] = masks[r]
        in_maps.append(im)
    return in_maps


LAST_RESULT = None


def kernel(**inputs):
    global LAST_RESULT
    key = ("prog", NLAYERS)
    if key not in _CACHE:
        _CACHE[key] = build_program(NLAYERS)
    nc = _CACHE[key]
    in_maps = prepare_inputs(inputs, NLAYERS)
    kw = {}
    if os.environ.get("BB_TRACE"):
        kw = dict(trace=True, trace_cores=[0])
    res = run_bass_kernel_spmd(nc, in_maps, core_ids=list(range(8)), **kw)
    LAST_RESULT = res
    outs = []
    for g in range(2):
        cols = []
        for r in range(4):
            o = res.results[g * 4 + r]["out"]          # [128, 6*T]
            o = o.reshape(128, DT, T).transpose(1, 0, 2).reshape(768, T)
            cols.append(o)
        xt = np.concatenate(cols, axis=1)              # [768, 1088]
        outs.append(xt[:, :SEQ].T)                     # [1025, 768]
    return np.stack(outs, axis=0).astype(np.float32)


if __name__ == "__main__":
    import reference
    ins = {k: np.asarray(v) for k, v in reference.setup_inputs().items()}
    got = kernel(**ins)
    print("kernel output", got.shape)
